# revision 21
# baseline (speedup 1.0000x reference)
"""Trainium2 Bass kernel for nn_CausalSelfAttention_61795989455492.

Sharding (8 cores): core c -> batch b = c//2, head-group hg = c%2 (8 of 16
heads). Each core runs QKV projection (its head slice), rotary, sliding-window
attention with joint prefix softmax, and a partial output projection over its
512 channel columns. Host sums the two partials per batch (pair reduce).

v3 design notes:
  - all staged inputs bf16; all matmuls bf16-moving; PSUM accumulates f32
  - rotary shuffle via PE permutation matmul; rotary combine split DVE/gpsimd
  - window mask = 0/1 bf16 multiplies on the 6 triangle blocks per chunk
    (alternating DVE/gpsimd per chunk)
  - per-head K/V/prefix loads replaced by single big host-pretransposed DMAs
    issued on gpsimd; w/x DMAs issued first on sync; out DMAs on scalar
  - exp: one scalar activation per chunk via a (p, 3, 384) strided PSUM view
  - denominators: per-head reciprocal_approx_fast + ones-broadcast matmul
"""

import sys
from contextlib import ExitStack

import numpy as np

sys.path.insert(0, "/opt/trn_rl_repo")

import ml_dtypes  # noqa: E402
import concourse.bass as bass  # noqa: E402
import concourse.tile as tile_mod  # noqa: E402
from concourse import bacc  # noqa: E402
from concourse import mybir  # noqa: E402

B, T, C, H, D = 4, 512, 1024, 16, 64
S_PREV, PFX, WINDOW = 1536, 256, 256
ROPE_BASE = 10000.0
HPC = 8  # heads per core
NCORES = 8

f32 = mybir.dt.float32
bf16 = mybir.dt.bfloat16

# window geometry per 512-col KV chunk, transposed layout:
# s-block tj (keys [128tj, 128tj+128)) attends t-run [T0[tj], T0[tj]+TN[tj])
_T0 = [0, 128, 256, 384]
_TN = [384, 384, 256, 128]
# psum column offset of each tj window inside the (128,1536) chunk tile
# (each matmul output must stay within one 512-col PSUM bank)
_POFF = [0, 512, 1024, 1280]
# exp-output column offset of each tj window inside the (128,1152) tile
_EOFF = [0, 384, 768, 1024]

# bf16 const block column offsets: [perm 128 | mask 1152 | onescol 8 | ones 64]
_CA_PERM = 0
_CA_M = 128
_CA_OC = 1280
_CA_ONES = 1288


def _emit(nc, tc, io):
    ctx = ExitStack()
    with ctx:
        const = ctx.enter_context(tc.tile_pool(name="const", bufs=1))
        big = ctx.enter_context(tc.tile_pool(name="big", bufs=1))
        qkrot = ctx.enter_context(tc.tile_pool(name="qkrot", bufs=1))
        vsb = ctx.enter_context(tc.tile_pool(name="vsb", bufs=1))
        ysb = ctx.enter_context(tc.tile_pool(name="ysb", bufs=1))
        nrm = ctx.enter_context(tc.tile_pool(name="nrm", bufs=1))
        exp_p = ctx.enter_context(tc.tile_pool(name="expsb", bufs=3))
        expp_p = ctx.enter_context(tc.tile_pool(name="exppref", bufs=1))

        # ---- phase 1 pools + DMAs: w/x interleaved first on sync ----
        with tc.tile_pool(name="wqkv", bufs=1) as wpool, \
             tc.tile_pool(name="xt", bufs=1) as xpool, \
             tc.tile_pool(name="ptmp", bufs=2) as ptmp, \
             tc.tile_pool(name="projps", bufs=4, space="PSUM") as projps, \
             tc.tile_pool(name="shps", bufs=2, space="PSUM") as shps:
            k_g = [big.tile([128, 1536], bf16, name=f"kg{g}", tag=f"kg{g}")
                   for g in range(4)]
            v_h = [big.tile([128, 12, 65], bf16, name=f"vh{i}", tag=f"vh{i}")
                   for i in range(HPC)]
            pref_h = [big.tile([128, 1024], bf16, name=f"prefh{i}", tag=f"prefh{i}")
                      for i in range(HPC)]
            cvn_h = [big.tile([128, 2, 65], bf16, name=f"cvnh{i}", tag=f"cvnh{i}")
                     for i in range(HPC)]
            xbig = xpool.tile([128, 8, 512], bf16, name="xbig", tag="xbig")
            sb_w = [wpool.tile([128, 8, 128], bf16, name=f"w{m}", tag=f"w{m}")
                    for m in range(8)]
            wvbig = wpool.tile([128, 8, 512], bf16, name="wv", tag="wv")
            cB = const.tile([128, 1024], f32, name="cB", tag="cB")
            cA = const.tile([128, 1352], bf16, name="cA", tag="cA")
            wpbig = big.tile([128, 4, 1024], bf16, name="wpbig", tag="wpbig")

            # two-queue scheme: sync carries x/w (+pref/cvn interleaved),
            # gpsimd carries consts + k/v. Head-critical first on each.
            nc.sync.dma_start(out=xbig[:, 0:4, :], in_=io["xT"].ap()[:, 0:4])
            nc.sync.dma_start(out=sb_w[0], in_=io["w_qk"].ap()[0])
            nc.sync.dma_start(out=sb_w[1], in_=io["w_qk"].ap()[1])
            nc.sync.dma_start(out=xbig[:, 4:8, :], in_=io["xT"].ap()[:, 4:8])
            for m in range(2, 8):
                nc.sync.dma_start(out=sb_w[m], in_=io["w_qk"].ap()[m])
                if m % 2 == 1:
                    hh = m // 2 - 1
                    nc.sync.dma_start(out=pref_h[hh], in_=io["prefT"].ap()[:, hh])
                    nc.sync.dma_start(out=cvn_h[hh], in_=io["cache_v_n"].ap()[:, hh])
            nc.sync.dma_start(out=pref_h[2], in_=io["prefT"].ap()[:, 2])
            nc.sync.dma_start(out=cvn_h[2], in_=io["cache_v_n"].ap()[:, 2])
            nc.sync.dma_start(out=wvbig, in_=io["w_v"].ap())
            for hh in range(3, HPC):
                nc.sync.dma_start(out=pref_h[hh], in_=io["prefT"].ap()[:, hh])
                nc.sync.dma_start(out=cvn_h[hh], in_=io["cache_v_n"].ap()[:, hh])
            nc.sync.dma_start(out=wpbig, in_=io["w_projT"].ap())

            # consts + k/v on gpsimd (parallel issue path)
            nc.gpsimd.dma_start(out=cA, in_=io["cA"].ap())
            nc.gpsimd.dma_start(out=cB, in_=io["cB"].ap())
            for h in range(HPC):
                if h % 2 == 0:
                    nc.gpsimd.dma_start(out=k_g[h // 2], in_=io["kT_cache"].ap()[h // 2])
                nc.gpsimd.dma_start(out=v_h[h], in_=io["v_cache"].ap()[:, h])

            sb_cos = cB[:, 0:512]
            sb_sin = cB[:, 512:1024]
            sb_perm = cA[:, _CA_PERM:_CA_PERM + 128]
            maskM = cA[:, _CA_M:_CA_M + 1152]
            onescol = cA[:, _CA_OC:_CA_OC + 8]
            ones64 = cA[0:1, _CA_ONES:_CA_ONES + 64]

            q_rot = [qkrot.tile([128, 512], bf16, name=f"qrot{i}", tag=f"qrot{i}") for i in range(4)]
            k_rot = [qkrot.tile([128, 512], bf16, name=f"krot{i}", tag=f"krot{i}") for i in range(4)]
            v_sb = [vsb.tile([128, 8, 65], bf16, name=f"vsb{i}", tag=f"vsb{i}") for i in range(4)]

            # ---------------- phase 1: qkv projection + rotary ----------------
            # q^T and k^T m-tiles 0..7 (q: 0..3, k: 4..7), rotary fused after
            for m in range(8):
                ps = projps.tile([128, 512], f32, name="projps", tag="projps")
                for c in range(8):
                    nc.tensor.matmul(
                        ps,
                        lhsT=sb_w[m][:, c, :],
                        rhs=xbig[:, c, :],
                        start=(c == 0),
                        stop=(c == 7),
                    )
                rot = q_rot[m] if m < 4 else k_rot[m - 4]
                qsb = ptmp.tile([128, 512], bf16, name="qsb", tag="qsb")
                nc.vector.tensor_copy(qsb, ps)
                shp = shps.tile([128, 512], f32, name="shps", tag="shps")
                nc.tensor.matmul(shp, lhsT=sb_perm, rhs=qsb, start=True, stop=True)
                t1 = ptmp.tile([128, 512], f32, name="t1", tag="t1")
                nc.vector.tensor_mul(t1, shp, sb_sin)
                t2 = ptmp.tile([128, 512], f32, name="t2", tag="t2")
                nc.gpsimd.tensor_mul(t2, qsb, sb_cos)
                nc.vector.tensor_add(rot, t1, t2)

            # v natural: (t, head*65+d)
            for tb in range(4):
                ps = projps.tile([128, 512], f32, name="projps", tag="projps")
                for c in range(8):
                    nc.tensor.matmul(
                        ps,
                        lhsT=xbig[:, c, tb * 128:(tb + 1) * 128],
                        rhs=wvbig[:, c, :],
                        start=(c == 0),
                        stop=(c == 7),
                    )
                nc.vector.tensor_copy(
                    v_sb[tb][:, :, 0:64],
                    ps.rearrange("p (h d) -> p h d", d=64),
                )
                nc.vector.tensor_copy(v_sb[tb][:, :, 64:65], onescol.unsqueeze(2))

        ynum = [nrm.tile([64, 512], bf16, name=f"ynum{i}", tag=f"ynum{i}") for i in range(HPC)]
        den = [nrm.tile([1, 512], f32, name=f"den{i}", tag=f"den{i}") for i in range(HPC)]
        recipb = [nrm.tile([1, 512], bf16, name=f"recipb{i}", tag=f"recipb{i}") for i in range(HPC)]
        y_t = [ysb.tile([128, 512], bf16, name=f"ysb{i}", tag=f"ysb{i}") for i in range(4)]

        # ---------------- phase 2: attention per head ----------------
        with tc.tile_pool(name="attps", bufs=2, space="PSUM") as attps_p, \
             tc.tile_pool(name="yaug", bufs=2, space="PSUM") as yaug_p:
            # all prefix exps up front (scalar is otherwise idle in phase 1)
            expps = []
            for h in range(HPC):
                expp = expp_p.tile([128, 1024], bf16, name=f"exppref{h}", tag=f"exppref{h}")
                nc.scalar.activation(out=expp, in_=pref_h[h],
                                     func=mybir.ActivationFunctionType.Exp)
                expps.append(expp)
            for h in range(HPC):
                hrow = (h % 2) * 64
                mt = h // 2

                yps = yaug_p.tile([65, 512], f32, name="yaug", tag="yaug")
                expp = expps[h]

                for pb in range(2):
                    nc.tensor.matmul(
                        yps,
                        lhsT=cvn_h[h][:, pb, :],
                        rhs=expp[:, pb * 512:(pb + 1) * 512],
                        start=(pb == 0),
                        stop=False,
                        skip_group_check=True,
                    )

                for ck in range(4):
                    aps = attps_p.tile([128, 1536], f32, name="attps", tag="attps")
                    for tj in range(4):
                        t0, tn, off = _T0[tj], _TN[tj], _POFF[tj]
                        if ck < 3:
                            kblk = k_g[h // 2][hrow:hrow + 64,
                                               ck * 512 + tj * 128: ck * 512 + (tj + 1) * 128]
                        else:
                            kblk = k_rot[mt][hrow:hrow + 64, tj * 128:(tj + 1) * 128]
                        qs = q_rot[mt][hrow:hrow + 64, t0:t0 + tn]
                        nc.tensor.matmul(
                            aps[:, off:off + tn],
                            lhsT=kblk,
                            rhs=qs,
                            start=True,
                            stop=True,
                            skip_group_check=True,
                        )

                    # exp (scale=1/sqrt(D)); one instruction over 3x384 strided view
                    ex = exp_p.tile([128, 1152], bf16, name="expsb", tag="expsb")
                    exv = ex.rearrange("p (w c) -> p w c", c=384)
                    nc.scalar.activation(
                        out=exv,
                        in_=aps.rearrange("p (w c) -> p w c", c=512)[:, 0:3, 0:384],
                        func=mybir.ActivationFunctionType.Exp,
                        scale=0.125,
                    )
                    # window mask: one 0/1 multiply (alternating DVE/gpsimd)
                    eng = nc.vector if (h * 4 + ck) % 2 == 0 else nc.gpsimd
                    eng.tensor_mul(ex, ex, maskM)

                    # AV accumulate into y_aug
                    for tj in range(4):
                        t0, tn, eoff = _T0[tj], _TN[tj], _EOFF[tj]
                        if ck < 3:
                            vblk = v_h[h][:, ck * 4 + tj, :]
                        else:
                            vblk = v_sb[tj][:, h, :]
                        nc.tensor.matmul(
                            yps[:, t0:t0 + tn],
                            lhsT=vblk,
                            rhs=ex[:, eoff:eoff + tn],
                            start=False,
                            stop=(ck == 3 and tj == 3),
                            skip_group_check=True,
                        )

                # stash numerator + denominator; frees the PSUM bank quickly
                nc.vector.tensor_copy(ynum[h], yps[0:64, :])
                nc.vector.tensor_copy(den[h], yps[64:65, :])

        # ---------------- phase 3: normalize + output projection ----------------
        with tc.tile_pool(name="outsb", bufs=3) as out_p, \
             tc.tile_pool(name="rbps", bufs=2, space="PSUM") as rbps_p, \
             tc.tile_pool(name="cpps", bufs=3, space="PSUM") as cpps_p:
            for h in range(HPC):
                hrow = (h % 2) * 64
                mt = h // 2
                nc.vector.reciprocal_approx_fast(out=den[h], in_=den[h])
                nc.vector.tensor_copy(recipb[h], den[h])
                rbp = rbps_p.tile([64, 512], f32, name="rbp", tag="rbp")
                nc.tensor.matmul(rbp, lhsT=ones64, rhs=recipb[h], start=True, stop=True)
                nc.vector.tensor_mul(y_t[mt][hrow:hrow + 64, :], ynum[h], rbp)

            for tb in range(4):
                for ng in range(2):
                    cps = cpps_p.tile([128, 512], f32, name="cpps", tag="cpps")
                    for ct in range(4):
                        nc.tensor.matmul(
                            cps,
                            lhsT=y_t[ct][:, tb * 128:(tb + 1) * 128],
                            rhs=wpbig[:, ct, ng * 512:(ng + 1) * 512],
                            start=(ct == 0),
                            stop=(ct == 3),
                        )
                    ob = out_p.tile([128, 512], f32, name="outsb", tag="outsb")
                    nc.vector.tensor_copy(ob, cps)
                    oeng = nc.scalar if (tb * 2 + ng) % 2 == 0 else nc.sync
                    oeng.dma_start(
                        out=io["out"].ap()[tb * 128:(tb + 1) * 128, ng * 512:(ng + 1) * 512],
                        in_=ob,
                    )


def build_nc():
    nc = bacc.Bacc("TRN2", target_bir_lowering=False, debug=False)
    io = {}
    io["xT"] = nc.declare_dram_parameter("xT", [128, 8, 512], bf16, isOutput=False)
    io["w_qk"] = nc.declare_dram_parameter("w_qk", [8, 128, 8, 128], bf16, isOutput=False)
    io["w_v"] = nc.declare_dram_parameter("w_v", [128, 8, 512], bf16, isOutput=False)
    io["kT_cache"] = nc.declare_dram_parameter("kT_cache", [4, 128, 1536], bf16, isOutput=False)
    io["v_cache"] = nc.declare_dram_parameter("v_cache", [128, 8, 12, 65], bf16, isOutput=False)
    io["prefT"] = nc.declare_dram_parameter("prefT", [128, 8, 1024], bf16, isOutput=False)
    io["cache_v_n"] = nc.declare_dram_parameter("cache_v_n", [128, 8, 2, 65], bf16, isOutput=False)
    io["w_projT"] = nc.declare_dram_parameter("w_projT", [128, 4, 1024], bf16, isOutput=False)
    io["cB"] = nc.declare_dram_parameter("cB", [128, 1024], f32, isOutput=False)
    io["cA"] = nc.declare_dram_parameter("cA", [128, 1352], bf16, isOutput=False)
    io["out"] = nc.declare_dram_parameter("out", [512, 1024], f32, isOutput=True)

    with tile_mod.TileContext(nc) as tc:
        _emit(nc, tc, io)
    nc.finalize()
    return nc


def _rotary_tables(start_index):
    half = D // 2
    inv_freq = 1.0 / (ROPE_BASE ** (np.arange(half, dtype=np.float32) / half))
    pos = (float(start_index) + np.arange(T, dtype=np.float32))
    ang = inv_freq[:, None] * pos[None, :]  # (32, 512): [d, t]
    c = np.cos(ang, dtype=np.float32)
    s = np.sin(ang, dtype=np.float32)
    cos2 = np.tile(c, (4, 1))  # (128, 512)
    sin2 = np.tile(np.concatenate([-s, s], axis=0), (2, 1))  # (128, 512)
    return np.ascontiguousarray(cos2), np.ascontiguousarray(sin2)


def _mask_const():
    # (128, 1152) 0/1: ex layout [tj0 384 | tj1 384 | tj2 256 | tj3 128]
    m = np.zeros((128, 1152), dtype=np.float32)
    for tj in range(4):
        t0, tn, eoff = _T0[tj], _TN[tj], _EOFF[tj]
        p = np.arange(128)[:, None]
        t = t0 + np.arange(tn)[None, :]
        dd = t - (tj * 128 + p)
        m[:, eoff:eoff + tn] = ((dd >= 0) & (dd <= WINDOW)).astype(np.float32)
    return m


def _perm_const():
    sigma = np.concatenate([
        np.arange(32, 64), np.arange(0, 32),
        np.arange(96, 128), np.arange(64, 96),
    ])
    p = np.zeros((128, 128), dtype=np.float32)
    p[sigma, np.arange(128)] = 1.0
    return p


def _to_bf16(a):
    return np.ascontiguousarray(a).astype(ml_dtypes.bfloat16)


def make_in_maps(x, c_attn_w, c_proj_w, cached_k, cached_v, att_prefix, cache_v, start_index):
    cos2, sin2 = _rotary_tables(np.asarray(start_index).item())
    cB = np.ascontiguousarray(np.concatenate([cos2, sin2], axis=1))  # (128,1024)
    perm = _perm_const()
    cA = np.zeros((128, 1352), np.float32)
    cA[:, _CA_PERM:_CA_PERM + 128] = perm
    cA[:, _CA_M:_CA_M + 1152] = _mask_const()
    cA[:, _CA_OC:_CA_OC + 8] = 1.0
    cA[:, _CA_ONES:_CA_ONES + 64] = 1.0
    cA = cA.astype(ml_dtypes.bfloat16)

    in_maps = []
    for core in range(NCORES):
        b, hg = core // 2, core % 2
        hs = slice(hg * HPC, (hg + 1) * HPC)
        r0, r1 = hg * 512, (hg + 1) * 512
        wq = c_attn_w[r0:r1]
        wk = c_attn_w[C + r0:C + r1]
        wv = c_attn_w[2 * C + r0:2 * C + r1]
        w_qkvT = np.concatenate([wq, wk, wv], axis=0).T
        p = att_prefix[b, hs].transpose(0, 2, 1)  # (8, 256, 512)
        prefT = np.concatenate([p[:, :128], p[:, 128:]], axis=2)  # (8,128,1024)
        v_aug = np.concatenate(
            [cached_v[b, hs], np.ones((HPC, S_PREV, 1), np.float32)], axis=2)
        cvn_aug = np.concatenate(
            [cache_v[b, hs], np.ones((HPC, PFX, 1), np.float32)], axis=2)
        w_qk = w_qkvT[:, 0:1024].reshape(8, 128, 8, 128).transpose(2, 1, 0, 3)
        w_v = w_qkvT[:, 1024:1536].reshape(8, 128, 512).transpose(1, 0, 2)
        in_maps.append({
            "xT": _to_bf16(x[b].T.reshape(8, 128, 512).transpose(1, 0, 2)),
            "w_qk": _to_bf16(w_qk),
            "w_v": _to_bf16(w_v),
            "kT_cache": _to_bf16(
                cached_k[b, hs].transpose(0, 2, 1).reshape(HPC // 2, 128, 1536)),
            "v_cache": _to_bf16(
                v_aug.reshape(HPC, 12, 128, 65).transpose(2, 0, 1, 3)),
            "prefT": _to_bf16(prefT.transpose(1, 0, 2)),
            "cache_v_n": _to_bf16(
                cvn_aug.reshape(HPC, 2, 128, 65).transpose(2, 0, 1, 3)),
            "w_projT": _to_bf16(
                c_proj_w[:, r0:r1].T.reshape(4, 128, 1024).transpose(1, 0, 2)),
            "cB": cB,
            "cA": cA,
        })
    return in_maps


_NC_CACHE = {}


def kernel(x, c_attn_w, c_proj_w, cached_k, cached_v, att_prefix, cache_v, start_index):
    x = np.asarray(x, dtype=np.float32)
    c_attn_w = np.asarray(c_attn_w, dtype=np.float32)
    c_proj_w = np.asarray(c_proj_w, dtype=np.float32)
    cached_k = np.asarray(cached_k, dtype=np.float32)
    cached_v = np.asarray(cached_v, dtype=np.float32)
    att_prefix = np.asarray(att_prefix, dtype=np.float32)
    cache_v = np.asarray(cache_v, dtype=np.float32)

    if "nc" not in _NC_CACHE:
        _NC_CACHE["nc"] = build_nc()
    nc = _NC_CACHE["nc"]

    in_maps = make_in_maps(x, c_attn_w, c_proj_w, cached_k, cached_v,
                           att_prefix, cache_v, start_index)
    from concourse.bass_utils import run_bass_kernel_spmd
    res = run_bass_kernel_spmd(nc, in_maps, list(range(NCORES)))
    outs = res.results
    y = np.empty((B, T, C), dtype=np.float32)
    for b in range(B):
        y[b] = outs[2 * b]["out"] + outs[2 * b + 1]["out"]
    return y


# revision 23
# speedup vs baseline: 1.1347x; 1.1347x over previous
"""Trainium2 Bass kernel for nn_CausalSelfAttention_61795989455492.

Sharding (8 cores): core c -> batch b = c//2, head-group hg = c%2 (8 of 16
heads). Each core runs QKV projection (its head slice), rotary, sliding-window
attention with joint prefix softmax, and a partial output projection over its
512 channel columns. Host sums the two partials per batch (pair reduce).

v3 design notes:
  - all staged inputs bf16; all matmuls bf16-moving; PSUM accumulates f32
  - rotary shuffle via PE permutation matmul; rotary combine split DVE/gpsimd
  - window mask = 0/1 bf16 multiplies on the 6 triangle blocks per chunk
    (alternating DVE/gpsimd per chunk)
  - per-head K/V/prefix loads replaced by single big host-pretransposed DMAs
    issued on gpsimd; w/x DMAs issued first on sync; out DMAs on scalar
  - exp: one scalar activation per chunk via a (p, 3, 384) strided PSUM view
  - denominators: per-head reciprocal_approx_fast + ones-broadcast matmul
"""

import sys
from contextlib import ExitStack

import numpy as np

sys.path.insert(0, "/opt/trn_rl_repo")

import ml_dtypes  # noqa: E402
import concourse.bass as bass  # noqa: E402
import concourse.tile as tile_mod  # noqa: E402
from concourse import bacc  # noqa: E402
from concourse import mybir  # noqa: E402

B, T, C, H, D = 4, 512, 1024, 16, 64
S_PREV, PFX, WINDOW = 1536, 256, 256
ROPE_BASE = 10000.0
HPC = 8  # heads per core
NCORES = 8

f32 = mybir.dt.float32
bf16 = mybir.dt.bfloat16

# window geometry per 512-col KV chunk, transposed layout:
# s-block tj (keys [128tj, 128tj+128)) attends t-run [T0[tj], T0[tj]+TN[tj])
_T0 = [0, 128, 256, 384]
_TN = [384, 384, 256, 128]
# psum column offset of each tj window inside the (128,1536) chunk tile
# (each matmul output must stay within one 512-col PSUM bank)
_POFF = [0, 512, 1024, 1280]
# exp-output column offset of each tj window inside the (128,1152) tile
_EOFF = [0, 384, 768, 1024]

# bf16 const block column offsets: [perm 128 | mask 1152 | onescol 8 | ones 64]
_CA_PERM = 0
_CA_M = 128
_CA_OC = 1280
_CA_ONES = 1288


def _emit(nc, tc, io):
    ctx = ExitStack()
    with ctx:
        const = ctx.enter_context(tc.tile_pool(name="const", bufs=1))
        big = ctx.enter_context(tc.tile_pool(name="big", bufs=1))
        qkrot = ctx.enter_context(tc.tile_pool(name="qkrot", bufs=1))
        vsb = ctx.enter_context(tc.tile_pool(name="vsb", bufs=1))
        ysb = ctx.enter_context(tc.tile_pool(name="ysb", bufs=1))
        nrm = ctx.enter_context(tc.tile_pool(name="nrm", bufs=1))
        exp_p = ctx.enter_context(tc.tile_pool(name="expsb", bufs=3))
        expp_p = ctx.enter_context(tc.tile_pool(name="exppref", bufs=1))

        # ---- phase 1 pools + DMAs: w/x interleaved first on sync ----
        with tc.tile_pool(name="wqkv", bufs=1) as wpool, \
             tc.tile_pool(name="xt", bufs=1) as xpool, \
             tc.tile_pool(name="ptmp", bufs=2) as ptmp, \
             tc.tile_pool(name="projps", bufs=4, space="PSUM") as projps, \
             tc.tile_pool(name="shps", bufs=2, space="PSUM") as shps:
            k_g = [big.tile([128, 1536], bf16, name=f"kg{g}", tag=f"kg{g}")
                   for g in range(4)]
            v_h = [big.tile([128, 12, 65], bf16, name=f"vh{i}", tag=f"vh{i}")
                   for i in range(HPC)]
            pref_h = [big.tile([128, 1024], bf16, name=f"prefh{i}", tag=f"prefh{i}")
                      for i in range(HPC)]
            cvn_h = [big.tile([128, 2, 65], bf16, name=f"cvnh{i}", tag=f"cvnh{i}")
                     for i in range(HPC)]
            xbig = xpool.tile([128, 8, 512], bf16, name="xbig", tag="xbig")
            sb_w = [wpool.tile([128, 8, 128], bf16, name=f"w{m}", tag=f"w{m}")
                    for m in range(8)]
            wvbig = wpool.tile([128, 8, 512], bf16, name="wv", tag="wv")
            cB = const.tile([128, 1024], f32, name="cB", tag="cB")
            cA = const.tile([128, 1352], bf16, name="cA", tag="cA")
            wpbig = big.tile([128, 4, 1024], bf16, name="wpbig", tag="wpbig")

            # two-queue scheme: sync carries x/w (+pref/cvn interleaved),
            # gpsimd carries consts + k/v. Head-critical first on each.
            nc.sync.dma_start(out=xbig[:, 0:4, :], in_=io["xT"].ap()[:, 0:4])
            nc.sync.dma_start(out=sb_w[0], in_=io["w_qk"].ap()[0])
            nc.sync.dma_start(out=sb_w[1], in_=io["w_qk"].ap()[1])
            nc.sync.dma_start(out=xbig[:, 4:8, :], in_=io["xT"].ap()[:, 4:8])
            for m in range(2, 8):
                nc.sync.dma_start(out=sb_w[m], in_=io["w_qk"].ap()[m])
                if m % 2 == 1:
                    hh = m // 2 - 1
                    nc.sync.dma_start(out=pref_h[hh], in_=io["prefT"].ap()[:, hh])
                    nc.sync.dma_start(out=cvn_h[hh], in_=io["cache_v_n"].ap()[:, hh])
            nc.sync.dma_start(out=pref_h[2], in_=io["prefT"].ap()[:, 2])
            nc.sync.dma_start(out=cvn_h[2], in_=io["cache_v_n"].ap()[:, 2])
            nc.sync.dma_start(out=wvbig, in_=io["w_v"].ap())
            for hh in range(3, HPC):
                nc.sync.dma_start(out=pref_h[hh], in_=io["prefT"].ap()[:, hh])
                nc.sync.dma_start(out=cvn_h[hh], in_=io["cache_v_n"].ap()[:, hh])
            nc.sync.dma_start(out=wpbig, in_=io["w_projT"].ap())

            # consts + k/v on gpsimd (parallel issue path)
            nc.gpsimd.dma_start(out=cA, in_=io["cA"].ap())
            nc.gpsimd.dma_start(out=cB, in_=io["cB"].ap())
            for h in range(HPC):
                if h % 2 == 0:
                    nc.gpsimd.dma_start(out=k_g[h // 2], in_=io["kT_cache"].ap()[h // 2])
                nc.gpsimd.dma_start(out=v_h[h], in_=io["v_cache"].ap()[:, h])

            sb_cos = cB[:, 0:512]
            sb_sin = cB[:, 512:1024]
            sb_perm = cA[:, _CA_PERM:_CA_PERM + 128]
            maskM = cA[:, _CA_M:_CA_M + 1152]
            onescol = cA[:, _CA_OC:_CA_OC + 8]
            ones64 = cA[0:1, _CA_ONES:_CA_ONES + 64]

            q_rot = [qkrot.tile([128, 512], bf16, name=f"qrot{i}", tag=f"qrot{i}") for i in range(4)]
            k_rot = [qkrot.tile([128, 512], bf16, name=f"krot{i}", tag=f"krot{i}") for i in range(4)]
            v_sb = [vsb.tile([128, 8, 65], bf16, name=f"vsb{i}", tag=f"vsb{i}") for i in range(4)]

            # ---------------- phase 1: qkv projection + rotary ----------------
            # q^T and k^T m-tiles 0..7 (q: 0..3, k: 4..7), rotary fused after
            for m in range(8):
                ps = projps.tile([128, 512], f32, name="projps", tag="projps")
                for c in range(8):
                    nc.tensor.matmul(
                        ps,
                        lhsT=sb_w[m][:, c, :],
                        rhs=xbig[:, c, :],
                        start=(c == 0),
                        stop=(c == 7),
                    )
                rot = q_rot[m] if m < 4 else k_rot[m - 4]
                qsb = ptmp.tile([128, 512], bf16, name="qsb", tag="qsb")
                nc.vector.tensor_copy(qsb, ps)
                shp = shps.tile([128, 512], f32, name="shps", tag="shps")
                nc.tensor.matmul(shp, lhsT=sb_perm, rhs=qsb, start=True, stop=True)
                t1 = ptmp.tile([128, 512], f32, name="t1", tag="t1")
                nc.vector.tensor_mul(t1, shp, sb_sin)
                t2 = ptmp.tile([128, 512], f32, name="t2", tag="t2")
                nc.gpsimd.tensor_mul(t2, qsb, sb_cos)
                nc.vector.tensor_add(rot, t1, t2)

            # v natural: (t, head*65+d)
            for tb in range(4):
                ps = projps.tile([128, 512], f32, name="projps", tag="projps")
                for c in range(8):
                    nc.tensor.matmul(
                        ps,
                        lhsT=xbig[:, c, tb * 128:(tb + 1) * 128],
                        rhs=wvbig[:, c, :],
                        start=(c == 0),
                        stop=(c == 7),
                    )
                nc.vector.tensor_copy(
                    v_sb[tb][:, :, 0:64],
                    ps.rearrange("p (h d) -> p h d", d=64),
                )
                nc.vector.tensor_copy(v_sb[tb][:, :, 64:65], onescol.unsqueeze(2))

        ynum = [nrm.tile([64, 512], bf16, name=f"ynum{i}", tag=f"ynum{i}") for i in range(HPC)]
        den = [nrm.tile([1, 512], f32, name=f"den{i}", tag=f"den{i}") for i in range(HPC)]
        recipb = [nrm.tile([1, 512], bf16, name=f"recipb{i}", tag=f"recipb{i}") for i in range(HPC)]
        y_t = [ysb.tile([128, 512], bf16, name=f"ysb{i}", tag=f"ysb{i}") for i in range(4)]

        # ---------------- phase 2: attention per head ----------------
        with tc.tile_pool(name="attps", bufs=2, space="PSUM") as attps_p, \
             tc.tile_pool(name="yaug", bufs=2, space="PSUM") as yaug_p:
            # all prefix exps up front (scalar is otherwise idle in phase 1)
            expps = []
            for h in range(HPC):
                expp = expp_p.tile([128, 1024], bf16, name=f"exppref{h}", tag=f"exppref{h}")
                nc.scalar.activation(out=expp, in_=pref_h[h],
                                     func=mybir.ActivationFunctionType.Exp)
                expps.append(expp)
            for h in range(HPC):
                hrow = (h % 2) * 64
                mt = h // 2

                yps = yaug_p.tile([65, 512], f32, name="yaug", tag="yaug")
                expp = expps[h]

                for pb in range(2):
                    nc.tensor.matmul(
                        yps,
                        lhsT=cvn_h[h][:, pb, :],
                        rhs=expp[:, pb * 512:(pb + 1) * 512],
                        start=(pb == 0),
                        stop=False,
                        skip_group_check=True,
                    )

                for ck in range(4):
                    aps = attps_p.tile([128, 1536], f32, name="attps", tag="attps")
                    for tj in range(4):
                        t0, tn, off = _T0[tj], _TN[tj], _POFF[tj]
                        if ck < 3:
                            kblk = k_g[h // 2][hrow:hrow + 64,
                                               ck * 512 + tj * 128: ck * 512 + (tj + 1) * 128]
                        else:
                            kblk = k_rot[mt][hrow:hrow + 64, tj * 128:(tj + 1) * 128]
                        qs = q_rot[mt][hrow:hrow + 64, t0:t0 + tn]
                        nc.tensor.matmul(
                            aps[:, off:off + tn],
                            lhsT=kblk,
                            rhs=qs,
                            start=True,
                            stop=True,
                            skip_group_check=True,
                        )

                    # exp (scale=1/sqrt(D)); one instruction over 3x384 strided view
                    ex = exp_p.tile([128, 1152], bf16, name="expsb", tag="expsb")
                    exv = ex.rearrange("p (w c) -> p w c", c=384)
                    nc.scalar.activation(
                        out=exv,
                        in_=aps.rearrange("p (w c) -> p w c", c=512)[:, 0:3, 0:384],
                        func=mybir.ActivationFunctionType.Exp,
                        scale=0.125,
                    )
                    # window mask: one 0/1 multiply on DVE
                    nc.vector.tensor_mul(ex, ex, maskM)

                    # AV accumulate into y_aug
                    for tj in range(4):
                        t0, tn, eoff = _T0[tj], _TN[tj], _EOFF[tj]
                        if ck < 3:
                            vblk = v_h[h][:, ck * 4 + tj, :]
                        else:
                            vblk = v_sb[tj][:, h, :]
                        nc.tensor.matmul(
                            yps[:, t0:t0 + tn],
                            lhsT=vblk,
                            rhs=ex[:, eoff:eoff + tn],
                            start=False,
                            stop=(ck == 3 and tj == 3),
                            skip_group_check=True,
                        )

                # stash numerator + denominator; frees the PSUM bank quickly
                nc.vector.tensor_copy(ynum[h], yps[0:64, :])
                nc.vector.tensor_copy(den[h], yps[64:65, :])

        # ---------------- phase 3: normalize + output projection ----------------
        with tc.tile_pool(name="outsb", bufs=3) as out_p, \
             tc.tile_pool(name="rbps", bufs=2, space="PSUM") as rbps_p, \
             tc.tile_pool(name="cpps", bufs=3, space="PSUM") as cpps_p:
            for h in range(HPC):
                hrow = (h % 2) * 64
                mt = h // 2
                nc.vector.reciprocal_approx_fast(out=den[h], in_=den[h])
                nc.vector.tensor_copy(recipb[h], den[h])
                rbp = rbps_p.tile([64, 512], f32, name="rbp", tag="rbp")
                nc.tensor.matmul(rbp, lhsT=ones64, rhs=recipb[h], start=True, stop=True)
                nc.vector.tensor_mul(y_t[mt][hrow:hrow + 64, :], ynum[h], rbp)

            for tb in range(4):
                for ng in range(2):
                    cps = cpps_p.tile([128, 512], f32, name="cpps", tag="cpps")
                    for ct in range(4):
                        nc.tensor.matmul(
                            cps,
                            lhsT=y_t[ct][:, tb * 128:(tb + 1) * 128],
                            rhs=wpbig[:, ct, ng * 512:(ng + 1) * 512],
                            start=(ct == 0),
                            stop=(ct == 3),
                        )
                    ob = out_p.tile([128, 512], f32, name="outsb", tag="outsb")
                    nc.vector.tensor_copy(ob, cps)
                    oeng = nc.scalar if (tb * 2 + ng) % 2 == 0 else nc.sync
                    oeng.dma_start(
                        out=io["out"].ap()[tb * 128:(tb + 1) * 128, ng * 512:(ng + 1) * 512],
                        in_=ob,
                    )


def build_nc():
    nc = bacc.Bacc("TRN2", target_bir_lowering=False, debug=False)
    io = {}
    io["xT"] = nc.declare_dram_parameter("xT", [128, 8, 512], bf16, isOutput=False)
    io["w_qk"] = nc.declare_dram_parameter("w_qk", [8, 128, 8, 128], bf16, isOutput=False)
    io["w_v"] = nc.declare_dram_parameter("w_v", [128, 8, 512], bf16, isOutput=False)
    io["kT_cache"] = nc.declare_dram_parameter("kT_cache", [4, 128, 1536], bf16, isOutput=False)
    io["v_cache"] = nc.declare_dram_parameter("v_cache", [128, 8, 12, 65], bf16, isOutput=False)
    io["prefT"] = nc.declare_dram_parameter("prefT", [128, 8, 1024], bf16, isOutput=False)
    io["cache_v_n"] = nc.declare_dram_parameter("cache_v_n", [128, 8, 2, 65], bf16, isOutput=False)
    io["w_projT"] = nc.declare_dram_parameter("w_projT", [128, 4, 1024], bf16, isOutput=False)
    io["cB"] = nc.declare_dram_parameter("cB", [128, 1024], f32, isOutput=False)
    io["cA"] = nc.declare_dram_parameter("cA", [128, 1352], bf16, isOutput=False)
    io["out"] = nc.declare_dram_parameter("out", [512, 1024], f32, isOutput=True)

    with tile_mod.TileContext(nc) as tc:
        _emit(nc, tc, io)
    nc.finalize()
    return nc


def _rotary_tables(start_index):
    half = D // 2
    inv_freq = 1.0 / (ROPE_BASE ** (np.arange(half, dtype=np.float32) / half))
    pos = (float(start_index) + np.arange(T, dtype=np.float32))
    ang = inv_freq[:, None] * pos[None, :]  # (32, 512): [d, t]
    c = np.cos(ang, dtype=np.float32)
    s = np.sin(ang, dtype=np.float32)
    cos2 = np.tile(c, (4, 1))  # (128, 512)
    sin2 = np.tile(np.concatenate([-s, s], axis=0), (2, 1))  # (128, 512)
    return np.ascontiguousarray(cos2), np.ascontiguousarray(sin2)


def _mask_const():
    # (128, 1152) 0/1: ex layout [tj0 384 | tj1 384 | tj2 256 | tj3 128]
    m = np.zeros((128, 1152), dtype=np.float32)
    for tj in range(4):
        t0, tn, eoff = _T0[tj], _TN[tj], _EOFF[tj]
        p = np.arange(128)[:, None]
        t = t0 + np.arange(tn)[None, :]
        dd = t - (tj * 128 + p)
        m[:, eoff:eoff + tn] = ((dd >= 0) & (dd <= WINDOW)).astype(np.float32)
    return m


def _perm_const():
    sigma = np.concatenate([
        np.arange(32, 64), np.arange(0, 32),
        np.arange(96, 128), np.arange(64, 96),
    ])
    p = np.zeros((128, 128), dtype=np.float32)
    p[sigma, np.arange(128)] = 1.0
    return p


def _to_bf16(a):
    return np.ascontiguousarray(a).astype(ml_dtypes.bfloat16)


def make_in_maps(x, c_attn_w, c_proj_w, cached_k, cached_v, att_prefix, cache_v, start_index):
    cos2, sin2 = _rotary_tables(np.asarray(start_index).item())
    cB = np.ascontiguousarray(np.concatenate([cos2, sin2], axis=1))  # (128,1024)
    perm = _perm_const()
    cA = np.zeros((128, 1352), np.float32)
    cA[:, _CA_PERM:_CA_PERM + 128] = perm
    cA[:, _CA_M:_CA_M + 1152] = _mask_const()
    cA[:, _CA_OC:_CA_OC + 8] = 1.0
    cA[:, _CA_ONES:_CA_ONES + 64] = 1.0
    cA = cA.astype(ml_dtypes.bfloat16)

    in_maps = []
    for core in range(NCORES):
        b, hg = core // 2, core % 2
        hs = slice(hg * HPC, (hg + 1) * HPC)
        r0, r1 = hg * 512, (hg + 1) * 512
        wq = c_attn_w[r0:r1]
        wk = c_attn_w[C + r0:C + r1]
        wv = c_attn_w[2 * C + r0:2 * C + r1]
        w_qkvT = np.concatenate([wq, wk, wv], axis=0).T
        p = att_prefix[b, hs].transpose(0, 2, 1)  # (8, 256, 512)
        prefT = np.concatenate([p[:, :128], p[:, 128:]], axis=2)  # (8,128,1024)
        v_aug = np.concatenate(
            [cached_v[b, hs], np.ones((HPC, S_PREV, 1), np.float32)], axis=2)
        cvn_aug = np.concatenate(
            [cache_v[b, hs], np.ones((HPC, PFX, 1), np.float32)], axis=2)
        w_qk = w_qkvT[:, 0:1024].reshape(8, 128, 8, 128).transpose(2, 1, 0, 3)
        w_v = w_qkvT[:, 1024:1536].reshape(8, 128, 512).transpose(1, 0, 2)
        in_maps.append({
            "xT": _to_bf16(x[b].T.reshape(8, 128, 512).transpose(1, 0, 2)),
            "w_qk": _to_bf16(w_qk),
            "w_v": _to_bf16(w_v),
            "kT_cache": _to_bf16(
                cached_k[b, hs].transpose(0, 2, 1).reshape(HPC // 2, 128, 1536)),
            "v_cache": _to_bf16(
                v_aug.reshape(HPC, 12, 128, 65).transpose(2, 0, 1, 3)),
            "prefT": _to_bf16(prefT.transpose(1, 0, 2)),
            "cache_v_n": _to_bf16(
                cvn_aug.reshape(HPC, 2, 128, 65).transpose(2, 0, 1, 3)),
            "w_projT": _to_bf16(
                c_proj_w[:, r0:r1].T.reshape(4, 128, 1024).transpose(1, 0, 2)),
            "cB": cB,
            "cA": cA,
        })
    return in_maps


_NC_CACHE = {}


def kernel(x, c_attn_w, c_proj_w, cached_k, cached_v, att_prefix, cache_v, start_index):
    x = np.asarray(x, dtype=np.float32)
    c_attn_w = np.asarray(c_attn_w, dtype=np.float32)
    c_proj_w = np.asarray(c_proj_w, dtype=np.float32)
    cached_k = np.asarray(cached_k, dtype=np.float32)
    cached_v = np.asarray(cached_v, dtype=np.float32)
    att_prefix = np.asarray(att_prefix, dtype=np.float32)
    cache_v = np.asarray(cache_v, dtype=np.float32)

    if "nc" not in _NC_CACHE:
        _NC_CACHE["nc"] = build_nc()
    nc = _NC_CACHE["nc"]

    in_maps = make_in_maps(x, c_attn_w, c_proj_w, cached_k, cached_v,
                           att_prefix, cache_v, start_index)
    from concourse.bass_utils import run_bass_kernel_spmd
    res = run_bass_kernel_spmd(nc, in_maps, list(range(NCORES)))
    outs = res.results
    y = np.empty((B, T, C), dtype=np.float32)
    for b in range(B):
        y[b] = outs[2 * b]["out"] + outs[2 * b + 1]["out"]
    return y
